# revision 10
# baseline (speedup 1.0000x reference)
"""GRU (MLP -> GRU-with-reset scan -> linear head) on 8 trn2 NeuronCores.

Strategy
--------
Batch is sharded across the 8 cores (16 rows each).  The key structural
trick: ``is_init`` resets the hidden state to zero (h *= 1-m) at ~50% of
(b, t) positions, so the scan decomposes into *independent segments*
between resets.  The host extracts the segments, packs them into
NL=512 parallel "lanes" (LPT bin packing), and permutes the (b, t)
columns of x accordingly.  The device then runs only S ~= 33 wide
recurrence steps over [128, 512] tiles instead of 1024 narrow ones.

Phase A computes the input-side gate projections for all columns
(column-pointwise, hence permutation-invariant):
    mlp1 = elu(W1 x + b1); mlp2 = elu(W2 mlp1 + b2)
    xg_g = Wih_g mlp2 + (b_ih_g [+ b_hh_g for g in r,z])
stored transposed+permuted in SBUF as bf16 [128, S, 512].

Phase B runs the scan per virtual step s over two half-groups of 256
lanes (two independent recurrences interleaved to hide chain latency):
    he = h * kb[s]                  (kb=0 where a new segment starts)
    hr|hz = Whr|Whz @ he  (+ xg preloaded in PSUM, accumulated by PE)
    r = sigmoid(.); z = sigmoid(.)
    pn = (Whn @ he + bhn) * r + xg_n;  n = tanh(pn)
    h' = n + z * (he - n)
    out[:, s] = Wout @ h' + bout
All matmuls use float32r APs (full PE rate at >=256 moving columns).
"""

import os
import sys

sys.path.insert(0, "/opt/trn_rl_repo")

import numpy as np
import ml_dtypes

import concourse.bacc as bacc
import concourse.bass as bass
import concourse.mybir as mybir
import concourse.tile as tile
from concourse.bass_utils import run_bass_kernel_spmd

B, T, IN, H, DIM = 128, 1024, 64, 128, 64
NCORES = 8
BC = B // NCORES          # batch rows per core
NL = 512                  # lanes per core
NG = 2                    # half-groups (independent recurrences)
GC = NL // NG             # columns per group

F32 = mybir.dt.float32
F32R = mybir.dt.float32r
BF16 = mybir.dt.bfloat16
AF = mybir.ActivationFunctionType
ALU = mybir.AluOpType

# stashed by kernel() for test harnesses
last_results = None


# --------------------------------------------------------------------------
# host-side packing
# --------------------------------------------------------------------------

def _pack_core(ib):
    """ib: [BC, T] bool (is_init). Returns (lanes, load) where lanes[l] is a
    list of (t0, length, b, use_hx) and load[l] the total length."""
    import heapq

    segs = []
    for b in range(BC):
        r = np.flatnonzero(ib[b])
        starts = np.unique(np.concatenate([[0], r]))
        ends = np.append(starts[1:], T)
        for t0, t1 in zip(starts, ends):
            use_hx = (t0 == 0) and (not bool(ib[b, 0]))
            segs.append((int(t0), int(t1 - t0), b, use_hx))

    lanes = [[] for _ in range(NL)]
    load = [0] * NL
    hx_segs = [s for s in segs if s[3]]
    others = sorted((s for s in segs if not s[3]), key=lambda s: -s[1])
    for i, s in enumerate(hx_segs):       # <= BC of these; must be lane-first
        lanes[i].append(s)
        load[i] += s[1]
    heap = [(load[l], l) for l in range(NL)]
    heapq.heapify(heap)
    for s in others:
        ld, l = heapq.heappop(heap)
        lanes[l].append(s)
        heapq.heappush(heap, (ld + s[1], l))
    return lanes, [sum(seg[1] for seg in lane) for lane in lanes]


def _build_core_arrays(xb, ib, hxb, lanes, S):
    """Build permuted x, kb mask, h0, position maps for one core."""
    N = S * NL
    perm_src = np.zeros(N, np.int64)
    valid = np.zeros(N, bool)
    kb = np.zeros((S, NL), np.float32)
    h0 = np.zeros((NL, H), np.float32)
    pos = np.full((BC, T), -1, np.int64)
    ht_cap = {}  # b -> (s, lane)

    for l, lane in enumerate(lanes):
        q = 0
        for (t0, ln, b, use_hx) in lane:
            cols = (np.arange(q, q + ln)) * NL + l
            perm_src[cols] = b * T + t0 + np.arange(ln)
            valid[cols] = True
            kb[q:q + ln, l] = 1.0
            if not use_hx:
                kb[q, l] = 0.0
            else:
                h0[l] = hxb[b]
            pos[b, t0:t0 + ln] = cols
            if t0 + ln == T:
                ht_cap[b] = (q + ln - 1, l)
            q += ln

    x_perm = np.zeros((N, IN), np.float32)
    x_perm[valid] = xb.reshape(BC * T, IN)[perm_src[valid]]
    xT = np.ascontiguousarray(x_perm.T)                      # [IN, N]
    kb_flat = kb.reshape(1, N).astype(ml_dtypes.bfloat16)    # [1, N]
    h0T = np.ascontiguousarray(h0.T)                         # [H, NL]
    return xT, kb_flat, h0T, pos, ht_cap


# --------------------------------------------------------------------------
# device program
# --------------------------------------------------------------------------

def _dram_bcast_ap(t):
    """Partition-stride-0 broadcast AP of a [1, N] DRAM tensor to 128 parts."""
    base = t[0:1, :]
    return bass.AP(tensor=base.tensor, offset=base.offset,
                   ap=[[0, 128]] + list(base.ap[1:]))


def _elu(nc, pool, psum, b_ap, bm1_ap, out):
    """out = elu(psum + b) using exp(min(u,0)) == min(exp(u), 1):
       elu(u) = max(u-1, -1) + min(exp(u), 1)."""
    texp = pool.tile([H, psum.shape[-1]], F32, tag="elu_exp")
    nc.scalar.activation(texp, psum, AF.Exp, bias=b_ap, scale=1.0)
    trelu = pool.tile([H, psum.shape[-1]], F32, tag="elu_relu")
    nc.vector.tensor_scalar(trelu, psum, bm1_ap, -1.0, ALU.add, ALU.max)
    nc.vector.scalar_tensor_tensor(out, texp, 1.0, trelu, ALU.min, ALU.add)


def _build_nc(S, ht_caps, n_slots):
    """Build the Bass program for S virtual steps.

    ht_caps: {(s, g): [(dst_slot, local_col), ...]} hidden-state captures.
    The capture schedule is the union across cores (the SPMD program is
    shared); each core writes every union slot and the host reads only the
    slots belonging to that core's own batch rows.
    """
    N = S * NL
    nc = bacc.Bacc(None, target_bir_lowering=False)

    xT = nc.dram_tensor("xT", [IN, N], F32R, kind="ExternalInput")
    kb = nc.dram_tensor("kb", [1, N], BF16, kind="ExternalInput")
    h0 = nc.dram_tensor("h0", [H, NL], F32, kind="ExternalInput")
    w1t = nc.dram_tensor("W1T", [IN, H], F32R, kind="ExternalInput")
    w2t = nc.dram_tensor("W2T", [H, H], F32R, kind="ExternalInput")
    wihr = nc.dram_tensor("WihrT", [H, H], F32R, kind="ExternalInput")
    wihz = nc.dram_tensor("WihzT", [H, H], F32R, kind="ExternalInput")
    wihn = nc.dram_tensor("WihnT", [H, H], F32R, kind="ExternalInput")
    whr = nc.dram_tensor("WhrT", [H, H], F32R, kind="ExternalInput")
    whz = nc.dram_tensor("WhzT", [H, H], F32R, kind="ExternalInput")
    whn = nc.dram_tensor("WhnT", [H, H], F32R, kind="ExternalInput")
    wout = nc.dram_tensor("WoutT", [H, DIM], F32R, kind="ExternalInput")
    b1 = nc.dram_tensor("b1", [H, 2], F32, kind="ExternalInput")   # [b, b-1]
    b2 = nc.dram_tensor("b2", [H, 2], F32, kind="ExternalInput")
    bxr = nc.dram_tensor("bxr", [H, 1], F32, kind="ExternalInput")
    bxz = nc.dram_tensor("bxz", [H, 1], F32, kind="ExternalInput")
    bxn = nc.dram_tensor("bxn", [H, 1], F32, kind="ExternalInput")
    bhn = nc.dram_tensor("bhn", [H, 1], F32, kind="ExternalInput")
    bout = nc.dram_tensor("bout", [DIM, 1], F32, kind="ExternalInput")

    outT = nc.dram_tensor("outT", [DIM, N], F32, kind="ExternalOutput")
    hTT = nc.dram_tensor("hTT", [H, n_slots], F32, kind="ExternalOutput")

    with tile.TileContext(nc) as tc:
        with tc.tile_pool(name="const", bufs=1) as cp:
            def cload(dram, shape, tag, dt=F32):
                t = cp.tile(shape, dt, tag=tag)
                nc.sync.dma_start(out=t, in_=dram[:, :])
                return t

            w1t_s = cload(w1t, [IN, H], "w1t", F32R)
            w2t_s = cload(w2t, [H, H], "w2t", F32R)
            wihr_s = cload(wihr, [H, H], "wihr", F32R)
            wihz_s = cload(wihz, [H, H], "wihz", F32R)
            wihn_s = cload(wihn, [H, H], "wihn", F32R)
            whr_s = cload(whr, [H, H], "whr", F32R)
            whz_s = cload(whz, [H, H], "whz", F32R)
            whn_s = cload(whn, [H, H], "whn", F32R)
            wout_s = cload(wout, [H, DIM], "wout", F32R)
            b1_s = cload(b1, [H, 2], "b1")
            b2_s = cload(b2, [H, 2], "b2")
            bxr_s = cload(bxr, [H, 1], "bxr")
            bxz_s = cload(bxz, [H, 1], "bxz")
            bxn_s = cload(bxn, [H, 1], "bxn")
            bhn_s = cload(bhn, [H, 1], "bhn")
            bout_s = cload(bout, [DIM, 1], "bout")

            xg_r = cp.tile([H, S, NL], BF16)
            xg_z = cp.tile([H, S, NL], BF16)
            xg_n = cp.tile([H, S, NL], BF16)
            kb_s = cp.tile([H, S, NL], BF16)
            h0_s = cp.tile([H, NL], F32)
            ht_s = cp.tile([H, n_slots], F32)

            nc.sync.dma_start(out=kb_s, in_=_dram_bcast_ap(kb))
            nc.sync.dma_start(out=h0_s, in_=h0[:, :])

            # ------------------------------------------------ Phase A
            with tc.tile_pool(name="xa", bufs=3) as xap, \
                 tc.tile_pool(name="mlp", bufs=3) as mlpp, \
                 tc.tile_pool(name="elu", bufs=3) as elup, \
                 tc.tile_pool(name="psA", bufs=3, space="PSUM") as psA:
                for s in range(S):
                    xt = xap.tile([IN, NL], F32R, tag="x")
                    nc.sync.dma_start(out=xt, in_=xT[:, s * NL:(s + 1) * NL])
                    p1 = psA.tile([H, NL], F32, tag="mm")
                    nc.tensor.matmul(p1, w1t_s, xt, start=True, stop=True)
                    m1 = mlpp.tile([H, NL], F32R, tag="m1")
                    _elu(nc, elup, p1, b1_s[:, 0:1], b1_s[:, 1:2], m1)

                    p2 = psA.tile([H, NL], F32, tag="mm")
                    nc.tensor.matmul(p2, w2t_s, m1, start=True, stop=True)
                    m2 = mlpp.tile([H, NL], F32R, tag="m2")
                    _elu(nc, elup, p2, b2_s[:, 0:1], b2_s[:, 1:2], m2)

                    for w_s, bg, xg in ((wihr_s, bxr_s, xg_r),
                                        (wihz_s, bxz_s, xg_z),
                                        (wihn_s, bxn_s, xg_n)):
                        pg = psA.tile([H, NL], F32, tag="mm")
                        nc.tensor.matmul(pg, w_s, m2, start=True, stop=True)
                        nc.scalar.activation(xg[:, s, :], pg, AF.Identity,
                                             bias=bg[:, 0:1], scale=1.0)

            # ------------------------------------------------ Phase B
            with tc.tile_pool(name="hp", bufs=3) as hp, \
                 tc.tile_pool(name="gp", bufs=2) as gp, \
                 tc.tile_pool(name="op", bufs=3) as op, \
                 tc.tile_pool(name="psRZ", bufs=1, space="PSUM") as psRZ, \
                 tc.tile_pool(name="psN", bufs=1, space="PSUM") as psN, \
                 tc.tile_pool(name="psO", bufs=2, space="PSUM") as psO:
                h_prev = [h0_s[:, g * GC:(g + 1) * GC] for g in range(NG)]
                for s in range(S):
                    for g in range(NG):
                        sl = slice(g * GC, (g + 1) * GC)
                        he = gp.tile([H, GC], F32R, tag=f"he{g}")
                        hp_ap = h_prev[g]
                        if hp_ap.dtype == F32R:
                            hp_ap = hp_ap.bitcast(F32)
                        nc.vector.tensor_tensor(he, hp_ap, kb_s[:, s, sl],
                                                ALU.mult)
                        prz = psRZ.tile([H, 2, GC], F32, tag=f"rz{g}")
                        # preload input-side r gate, PE accumulates on top
                        nc.scalar.activation(prz[:, 0, :], xg_r[:, s, sl],
                                             AF.Identity, bias=0.0, scale=1.0)
                        nc.tensor.matmul(prz[:, 0, :], whr_s, he, start=False,
                                         stop=True, skip_group_check=True)
                        nc.tensor.matmul(prz[:, 1, :], whz_s, he,
                                         start=True, stop=True)
                        nc.vector.tensor_tensor(prz[:, 1, :], prz[:, 1, :],
                                                xg_z[:, s, sl], ALU.add)
                        r_sb = gp.tile([H, GC], F32, tag=f"r{g}")
                        nc.scalar.activation(r_sb, prz[:, 0, :], AF.Sigmoid)
                        z_sb = gp.tile([H, GC], F32, tag=f"z{g}")
                        nc.scalar.activation(z_sb, prz[:, 1, :], AF.Sigmoid)

                        pn = psN.tile([H, GC], F32, tag=f"n{g}")
                        nc.tensor.matmul(pn, whn_s, he, start=True, stop=True)
                        nc.vector.scalar_tensor_tensor(pn, pn, bhn_s[:, 0:1],
                                                       r_sb, ALU.add, ALU.mult)
                        nc.vector.tensor_tensor(pn, pn, xg_n[:, s, sl], ALU.add)
                        n_sb = gp.tile([H, GC], F32, tag=f"n{g}s")
                        nc.scalar.activation(n_sb, pn, AF.Tanh)

                        d = gp.tile([H, GC], F32, tag=f"d{g}")
                        nc.vector.tensor_sub(d, he.bitcast(F32), n_sb)
                        zd = gp.tile([H, GC], F32, tag=f"zd{g}")
                        nc.vector.tensor_mul(zd, z_sb, d)
                        hn_t = hp.tile([H, GC], F32R, tag=f"h{g}")
                        nc.vector.tensor_add(hn_t, n_sb, zd)
                        h_prev[g] = hn_t

                        po = psO.tile([DIM, GC], F32, tag=f"o{g}")
                        nc.tensor.matmul(po, wout_s, hn_t, start=True,
                                         stop=True)
                        osb = op.tile([DIM, GC], F32, tag=f"ob{g}")
                        nc.scalar.activation(osb, po, AF.Identity,
                                             bias=bout_s[:, 0:1], scale=1.0)
                        nc.sync.dma_start(
                            out=outT[:, s * NL + g * GC: s * NL + (g + 1) * GC],
                            in_=osb)

                        for dst_col, local_col in ht_caps.get((s, g), ()):
                            nc.vector.tensor_copy(
                                ht_s[:, dst_col:dst_col + 1],
                                hn_t.bitcast(F32)[:, local_col:local_col + 1])
            nc.sync.dma_start(out=hTT[:, :], in_=ht_s)

    nc.finalize()
    return nc


# --------------------------------------------------------------------------
# entry point
# --------------------------------------------------------------------------

def kernel(x, is_init, hx, W1, b1, W2, b2, W_ih, b_ih, W_hh, b_hh, W_out,
           b_out):
    global last_results
    x = np.asarray(x, np.float32)
    ib_all = np.asarray(is_init).astype(bool)[..., 0]   # [B, T]
    hx = np.asarray(hx, np.float32)

    # ---- per-core packing (two passes: S must be global across cores)
    packs = []
    S = 0
    for c in range(NCORES):
        ib = ib_all[c * BC:(c + 1) * BC]
        lanes, load = _pack_core(ib)
        packs.append(lanes)
        S = max(S, max(load))

    per_core = []
    for c in range(NCORES):
        sl = slice(c * BC, (c + 1) * BC)
        xT, kb_f, h0T, pos, ht_cap = _build_core_arrays(
            x[sl], ib_all[sl], hx[sl], packs[c], S)
        per_core.append((xT, kb_f, h0T, pos, ht_cap))

    # hT captures are data-dependent per core, but the SPMD program is shared.
    # Emit the union of capture points; a capture that a core doesn't need
    # writes a garbage column that the host ignores... except every core DOES
    # need exactly BC captures at ITS OWN positions.  So instead emit captures
    # at the union of (s, g, local_col, dst_col) across cores -- wrong data
    # for other cores.  Solution: captures must be at identical positions.
    # They aren't, so we handle hT on the host from outT instead?  outT is
    # post-head.  Cheapest correct device path: every core captures at the
    # union of positions into distinct dst slots, host selects its own.
    cap_union = {}
    cap_slot = {}
    slot = 0
    for c in range(NCORES):
        ht_cap = per_core[c][4]
        for b, (s_pos, lane) in sorted(ht_cap.items()):
            key = (s_pos, lane)
            if key not in cap_slot:
                cap_slot[key] = slot
                g = lane // GC
                cap_union.setdefault((s_pos, g), []).append(
                    (cap_slot[key], lane - g * GC))
                slot += 1
    n_slots = slot

    nc = _build_nc_cached(S, _freeze_caps(cap_union), n_slots)

    W1 = np.asarray(W1, np.float32)
    W2 = np.asarray(W2, np.float32)
    W_ih = np.asarray(W_ih, np.float32)
    W_hh = np.asarray(W_hh, np.float32)
    W_out = np.asarray(W_out, np.float32)
    b1 = np.asarray(b1, np.float32)
    b2 = np.asarray(b2, np.float32)
    b_ih = np.asarray(b_ih, np.float32)
    b_hh = np.asarray(b_hh, np.float32)
    b_out = np.asarray(b_out, np.float32)

    shared = {
        "W1T": np.ascontiguousarray(W1.T),
        "W2T": np.ascontiguousarray(W2.T),
        "WihrT": np.ascontiguousarray(W_ih[0:H].T),
        "WihzT": np.ascontiguousarray(W_ih[H:2 * H].T),
        "WihnT": np.ascontiguousarray(W_ih[2 * H:3 * H].T),
        "WhrT": np.ascontiguousarray(W_hh[0:H].T),
        "WhzT": np.ascontiguousarray(W_hh[H:2 * H].T),
        "WhnT": np.ascontiguousarray(W_hh[2 * H:3 * H].T),
        "WoutT": np.ascontiguousarray(W_out.T),
        "b1": np.stack([b1, b1 - 1.0], 1).astype(np.float32),
        "b2": np.stack([b2, b2 - 1.0], 1).astype(np.float32),
        "bxr": (b_ih[0:H] + b_hh[0:H]).reshape(H, 1).astype(np.float32),
        "bxz": (b_ih[H:2 * H] + b_hh[H:2 * H]).reshape(H, 1).astype(np.float32),
        "bxn": b_ih[2 * H:3 * H].reshape(H, 1).astype(np.float32),
        "bhn": b_hh[2 * H:3 * H].reshape(H, 1).astype(np.float32),
        "bout": b_out.reshape(DIM, 1).astype(np.float32),
    }

    in_maps = []
    for c in range(NCORES):
        xT, kb_f, h0T, pos, ht_cap = per_core[c]
        m = dict(shared)
        m["xT"] = xT
        m["kb"] = kb_f
        m["h0"] = h0T
        in_maps.append(m)

    trace = os.environ.get("KERNEL_TRACE", "0") == "1"
    res = run_bass_kernel_spmd(nc, in_maps, core_ids=list(range(NCORES)),
                               trace=trace)
    last_results = res

    out = np.empty((B, T, DIM), np.float32)
    hT = np.empty((B, H), np.float32)
    for c in range(NCORES):
        _, _, _, pos, ht_cap = per_core[c]
        outT_c = res.results[c]["outT"]          # [DIM, N]
        hTT_c = res.results[c]["hTT"]            # [H, n_slots-padded BC]
        ot = outT_c.T                            # [N, DIM]
        out[c * BC:(c + 1) * BC] = ot[pos.reshape(-1)].reshape(BC, T, DIM)
        for b, key in sorted(ht_cap.items()):
            hT[c * BC + b] = hTT_c[:, cap_slot[key]]
    return out, hT


def _freeze_caps(cap_union):
    return tuple(sorted((k, tuple(sorted(v))) for k, v in cap_union.items()))


_build_cache = {}


def _build_nc_cached(S, caps_frozen, n_slots):
    key = (S, caps_frozen, n_slots)
    if key not in _build_cache:
        caps = {k: list(v) for k, v in caps_frozen}
        _build_cache[key] = _build_nc(S, caps, max(n_slots, 1))
    return _build_cache[key]
